# revision 21
# baseline (speedup 1.0000x reference)
"""DeepTEN encoding kernel for Trainium2 (8 NeuronCores, SPMD data-parallel over batch).

Math (per batch b):
    xf = x[b] viewed (D, N), N = H*W
    dist[n,k] = ||xf[:,n] - c[k]||^2 ;  logits = -scale * dist ;  A = softmax_k(logits)
    E[k,d] = sum_n A[n,k] * (xf[d,n] - c[k,d]) = (A^T X)[k,d] - colsum(A)[k]*c[k,d]

Device decomposition (softmax in (n-partitions, k-free) layout):
    w = -scale (>0), maxs = max(w)
    PSUM(xc)[n,k] = -2*w_k*<x_n,c_k> + w_k*csq_k      (x-tile-stationary matmuls +
                                                       a rank-1 seed matmul)
    G[n,k] = exp((w_k-maxs)*x_sq[n])                  (DVE mult + ACT exp; x_sq is
                                                       computed exactly on host, fp32)
    P = exp(PSUM) * G ;  S[n] = sum_k P ;  A = P / S
    (shifting logits by maxs*x_sq[n] bounds exp args; the gap to the true rowmax
     is < ~4 so S never underflows)
    psum_E[k, 0:D] += sum_n A[n,k]*xT[n,d]            (PE accumulates whole batch,
    psum_E[k, D]   += sum_n A[n,k]                     ones-column fused colsum)

x is uploaded twice in bf16 — (D,N) for the distance matmuls and pre-transposed
tiles (p, gi, d) for the aggregation matmuls — so no on-device transpose is needed;
total HBM traffic ~= one fp32 read of x. The mm2s of superblock s are emitted after
the softmax chain of superblock s+1 (software pipelining) and ping-pong between two
PSUM accumulators to avoid back-to-back accumulate stalls. Host does only layout /
dtype transforms of inputs plus the tiny (K,D)-level finishing arithmetic.
"""
import os
import sys
import numpy as np

sys.path.insert(0, "/opt/trn_rl_repo")

import ml_dtypes  # noqa: E402

BF16 = ml_dtypes.bfloat16

B, D, H, W = 32, 128, 128, 128
K = 32
N = H * W            # 16384
NCORES = 8
BPC = B // NCORES    # batches per core
TILN = 128           # n per tile (matmul stationary width)
NTIL = 16            # tiles per block
BLKN = TILN * NTIL   # 2048 n per block
NBLK = N // BLKN     # 8 blocks per batch

_CACHE = {}


def _build_module():
    from contextlib import ExitStack
    import concourse.tile as tile
    from concourse import bacc, mybir

    nc = bacc.Bacc("TRN2", target_bir_lowering=False, debug=False, num_devices=NCORES)
    bf = mybir.dt.bfloat16
    f32 = mybir.dt.float32

    x_d = nc.dram_tensor("x", [BPC, D, N], bf, kind="ExternalInput").ap()
    # xt[b, p, gi, d] = x[b, d, gi*128 + p]
    xt_d = nc.dram_tensor("xt", [BPC, 128, N // TILN, D + 1], bf, kind="ExternalInput").ap()
    # xsqc[b, p, sib, j] = x_sq[b, n],  n = sib*4096 + j*128 + p
    xsqc_d = nc.dram_tensor("xsqc", [BPC, 128, N // 4096, 32], f32, kind="ExternalInput").ap()
    # wmrep[p, j*K+k] = w[k] - maxs   (f32: bf16 would lose ~0.25 abs on wm*xsq)
    wmrep_d = nc.dram_tensor("wmrep", [128, 32 * K], f32, kind="ExternalInput").ap()
    # wcsqrep[0, j*K+k] = w[k]*csq[k]
    wcsqr_d = nc.dram_tensor("wcsqrep", [1, 32 * K], bf, kind="ExternalInput").ap()
    w1_d = nc.dram_tensor("w1", [D, K], bf, kind="ExternalInput").ap()
    oute_d = nc.dram_tensor("out_e", [BPC, K, 2, D + 1], f32, kind="ExternalOutput").ap()

    with tile.TileContext(nc) as tc, ExitStack() as ctx:
        cpool = ctx.enter_context(tc.tile_pool(name="const", bufs=1))
        xpool = ctx.enter_context(tc.tile_pool(name="xblk", bufs=4))
        xtpool = ctx.enter_context(tc.tile_pool(name="xtblk", bufs=4))
        gpool = ctx.enter_context(tc.tile_pool(name="gblk", bufs=3))
        qpool = ctx.enter_context(tc.tile_pool(name="xsqb", bufs=2))
        ppool = ctx.enter_context(tc.tile_pool(name="pexp", bufs=3))
        npool = ctx.enter_context(tc.tile_pool(name="pnorm", bufs=3))
        vpool = ctx.enter_context(tc.tile_pool(name="small", bufs=4))
        ps_xc = ctx.enter_context(tc.tile_pool(name="ps_xc", bufs=2, space="PSUM"))
        ps_e = ctx.enter_context(tc.tile_pool(name="ps_e", bufs=2, space="PSUM"))

        w1_sb = cpool.tile([D, K], bf)
        nc.sync.dma_start(out=w1_sb[:], in_=w1_d[:, :])
        wmrep_sb = cpool.tile([128, 32 * K], f32)
        nc.sync.dma_start(out=wmrep_sb[:], in_=wmrep_d[:, :])
        wcsqr_sb = cpool.tile([1, 32 * K], bf)
        nc.sync.dma_start(out=wcsqr_sb[:], in_=wcsqr_d[:, :])
        ones1_sb = cpool.tile([1, 128], bf)
        nc.vector.memset(ones1_sb[:], 1.0)

        NSUP = 2                 # blocks per superblock load
        SUPN = BLKN * NSUP       # 4096 n per load chunk
        NSB = NBLK // NSUP       # superblocks per batch
        TPS = NTIL * NSUP        # 32 tiles per superblock

        # Software pipeline: mm2s of superblock s are emitted after the
        # softmax chain of superblock s+1, so the PE hides the chain latency.
        pending = []  # (b, sup_in_batch, pn_sb, xt_sb)
        psum_es = {}
        xsq_bs = {}
        first_mm2 = {}

        def emit_mm2s(b, sib, pn_sb, xt_sb):
            pe0, pe1 = psum_es[b]
            ff = first_mm2[b]
            for i in range(TPS):
                pp = i % 2
                nc.tensor.matmul(
                    (pe0, pe1)[pp][:],
                    lhsT=pn_sb[:, K * i : K * (i + 1)],
                    rhs=xt_sb[:, i, :],
                    start=ff[pp],
                    stop=(sib == NSB - 1 and i >= TPS - 2),
                )
                ff[pp] = False
            if sib == NSB - 1:
                e_sb = vpool.tile([K, 2, D + 1], f32, tag="e_out")
                nc.vector.tensor_copy(e_sb[:, 0, :], pe0[:])
                nc.vector.tensor_copy(e_sb[:, 1, :], pe1[:])
                nc.sync.dma_start(out=oute_d[b], in_=e_sb[:])

        for gsup in range(BPC * NSB):
            b, sib = divmod(gsup, NSB)
            if sib == 0:
                xsq_b = qpool.tile([128, (N // 4096) * 32], f32, name=f"xsq_b{b}")
                nc.sync.dma_start(out=xsq_b[:], in_=xsqc_d[b].rearrange("p s j -> p (s j)"))
                xsq_bs[b] = xsq_b
                psum_es[b] = (
                    ps_e.tile([K, D + 1], f32, tag="pe0", name=f"psum_e0_b{b}"),
                    ps_e.tile([K, D + 1], f32, tag="pe1", name=f"psum_e1_b{b}"),
                )
                first_mm2[b] = [True, True]
            soff = sib * SUPN
            x_sb = xpool.tile([D, SUPN], bf)
            nc.sync.dma_start(out=x_sb[:], in_=x_d[b][:, soff : soff + SUPN])
            xt_sb = xtpool.tile([128, TPS, D + 1], bf)
            nc.scalar.dma_start(
                out=xt_sb[:], in_=xt_d[b][:, sib * TPS : (sib + 1) * TPS, :]
            )
            psum_xc = ps_xc.tile([128, TPS * K], f32)
            for h in range(2):
                nc.tensor.matmul(
                    psum_xc[:, 512 * h : 512 * (h + 1)],
                    lhsT=ones1_sb[:],
                    rhs=wcsqr_sb[:, 512 * h : 512 * (h + 1)],
                    start=True,
                    stop=False,
                    skip_group_check=True,
                )
            for i in range(TPS):
                nc.tensor.matmul(
                    psum_xc[:, K * i : K * (i + 1)],
                    lhsT=x_sb[:, TILN * i : TILN * (i + 1)],
                    rhs=w1_sb[:, :],
                    start=False,
                    stop=True,
                    skip_group_check=True,
                )

            pe_sb = ppool.tile([128, TPS * K], bf, tag="pexp")
            nc.scalar.activation(
                pe_sb[:], psum_xc[:], mybir.ActivationFunctionType.Exp
            )
            t1_sb = gpool.tile([128, TPS * K], f32, tag="t1")
            nc.vector.tensor_tensor(
                t1_sb[:].rearrange("p (j k) -> p j k", k=K),
                wmrep_sb[:].rearrange("p (j k) -> p j k", k=K),
                xsq_bs[b][:, sib * 32 : (sib + 1) * 32].broadcast_to([128, 32, K]),
                op=mybir.AluOpType.mult,
            )
            g_sb = gpool.tile([128, TPS * K], bf, tag="g")
            nc.scalar.activation(g_sb[:], t1_sb[:], mybir.ActivationFunctionType.Exp)
            p_sb = ppool.tile([128, TPS * K], bf, tag="p")
            nc.gpsimd.tensor_mul(p_sb[:], pe_sb[:], g_sb[:])
            p3 = p_sb[:].rearrange("p (i k) -> p i k", k=K)
            s_sb = vpool.tile([128, TPS], f32, tag="s")
            nc.vector.reduce_sum(s_sb[:], p3, axis=mybir.AxisListType.X)
            sinv_sb = vpool.tile([128, TPS], f32, tag="sinv")
            nc.vector.reciprocal(sinv_sb[:], s_sb[:])
            # Expand sinv to a packed bf16 tile on GPSIMD so the normalize
            # multiply below qualifies for the DVE 2-byte fast path.
            sexp_sb = npool.tile([128, TPS * K], bf, tag="sexp")
            nc.gpsimd.tensor_copy(
                sexp_sb[:].rearrange("p (j k) -> p j k", k=K),
                sinv_sb[:].broadcast_to([128, TPS, K]),
            )
            pn_sb = npool.tile([128, TPS * K], bf, tag="pn")
            nc.vector.tensor_mul(pn_sb[:], p_sb[:], sexp_sb[:])

            pending.append((b, sib, pn_sb, xt_sb))
            if len(pending) > 1:
                emit_mm2s(*pending.pop(0))

        while pending:
            emit_mm2s(*pending.pop(0))

    nc.compile()
    return nc


def _get_module():
    if "nc" not in _CACHE:
        _CACHE["nc"] = _build_module()
    return _CACHE["nc"]


def _host_prep(x, codewords, scale):
    x = np.asarray(x, dtype=np.float32)
    c = np.asarray(codewords, dtype=np.float32)
    s = np.asarray(scale, dtype=np.float32)

    w = -s                           # (K,) in (0, 1)
    maxs = float(w.max())
    w1 = (-2.0 * (w[:, None] * c)).T.astype(BF16)           # (D, K)
    wm = w - maxs                                           # (K,) <= 0
    wcsq = w * (c * c).sum(axis=1)                          # (K,)

    xf = x.reshape(B, D, N)
    xsq = np.einsum("bdn,bdn->bn", xf, xf)                  # (B, N) fp32
    # xsqc[b, p, s, j] = xsq[b, s*4096 + j*128 + p]
    xsqc = np.ascontiguousarray(
        xsq.reshape(B, N // 4096, 32, 128).transpose(0, 3, 1, 2)
    )                                                       # (B, 128, N/4096, 32) f32
    wmrep = np.ascontiguousarray(
        np.broadcast_to(np.tile(wm, 32)[None, :], (128, 32 * K))
    ).astype(np.float32)
    wcsqrep = np.tile(wcsq, 32)[None, :].astype(BF16)       # (1, 32*K)

    xb = xf.astype(BF16)                                    # (B, D, N)
    # xt[b, p, gi, d] = xf[b, d, gi*128 + p];  xt[..., D] = 1.0 (fused colsum column)
    xt = np.ones((B, N // TILN, TILN, D + 1), dtype=BF16)
    xt[:, :, :, :D] = xf.transpose(0, 2, 1).reshape(B, N // TILN, TILN, D).astype(BF16)
    xt = np.ascontiguousarray(xt.transpose(0, 2, 1, 3))     # (B, 128, N/128, D+1)
    return xb, xt, xsqc, wmrep, wcsqrep, w1


def make_in_maps(x, codewords, scale):
    xb, xt, xsqc, wmrep, wcsqrep, w1 = _host_prep(x, codewords, scale)
    in_maps = []
    for ci in range(NCORES):
        sl = slice(BPC * ci, BPC * (ci + 1))
        in_maps.append(
            {
                "x": np.ascontiguousarray(xb[sl]),
                "xt": np.ascontiguousarray(xt[sl]),
                "xsqc": np.ascontiguousarray(xsqc[sl]),
                "wmrep": wmrep,
                "wcsqrep": wcsqrep,
                "w1": w1,
            }
        )
    return in_maps


def finish_output(results, codewords):
    c = np.asarray(codewords, dtype=np.float32)
    out = np.zeros((B, K * D), dtype=np.float32)
    for ci, r in enumerate(results):
        for bb in range(BPC):
            e_parts = r["out_e"][bb][:, 0, :] + r["out_e"][bb][:, 1, :]   # (K, D+1)
            e = e_parts[:, :D] - e_parts[:, D : D + 1] * c
            out[BPC * ci + bb] = e.reshape(-1)
    return out


def kernel(x, codewords, scale):
    from concourse.bass_utils import run_bass_kernel_spmd
    from concourse.bass_interp import get_hw_module

    nc = _get_module()
    in_maps = make_in_maps(x, codewords, scale)

    old_m = nc.m
    nc.m = get_hw_module(nc.m)
    try:
        res = run_bass_kernel_spmd(nc, in_maps, core_ids=list(range(NCORES)))
    finally:
        nc.m = old_m
    return finish_output(res.results, codewords)


# revision 22
# speedup vs baseline: 1.4098x; 1.4098x over previous
"""DeepTEN encoding kernel for Trainium2 (8 NeuronCores, SPMD data-parallel over batch).

Math (per batch b):
    xf = x[b] viewed (D, N), N = H*W
    dist[n,k] = ||xf[:,n] - c[k]||^2 ;  logits = -scale * dist ;  A = softmax_k(logits)
    E[k,d] = sum_n A[n,k] * (xf[d,n] - c[k,d]) = (A^T X)[k,d] - colsum(A)[k]*c[k,d]

Device decomposition (softmax in (n-partitions, k-free) layout):
    w = -scale (>0), maxs = max(w)
    PSUM(xc)[n,k] = -2*w_k*<x_n,c_k> + w_k*csq_k      (x-tile-stationary matmuls +
                                                       a rank-1 seed matmul)
    G[n,k] = exp((w_k-maxs)*x_sq[n])                  (DVE mult + ACT exp; x_sq is
                                                       computed exactly on host, fp32)
    P = exp(PSUM) * G ;  S[n] = sum_k P ;  A = P / S
    (shifting logits by maxs*x_sq[n] bounds exp args; the gap to the true rowmax
     is < ~4 so S never underflows)
    psum_E[k, 0:D] += sum_n A[n,k]*xT[n,d]            (PE accumulates whole batch,
    psum_E[k, D]   += sum_n A[n,k]                     ones-column fused colsum)

x is uploaded twice in bf16 — (D,N) for the distance matmuls and pre-transposed
tiles (p, gi, d) for the aggregation matmuls — so no on-device transpose is needed;
total HBM traffic ~= one fp32 read of x. The mm2s of superblock s are emitted after
the softmax chain of superblock s+1 (software pipelining) and ping-pong between two
PSUM accumulators to avoid back-to-back accumulate stalls. Host does only layout /
dtype transforms of inputs plus the tiny (K,D)-level finishing arithmetic.
"""
import os
import sys
import numpy as np

sys.path.insert(0, "/opt/trn_rl_repo")

import ml_dtypes  # noqa: E402

BF16 = ml_dtypes.bfloat16

B, D, H, W = 32, 128, 128, 128
K = 32
N = H * W            # 16384
NCORES = 8
BPC = B // NCORES    # batches per core
TILN = 128           # n per tile (matmul stationary width)
NTIL = 16            # tiles per block
BLKN = TILN * NTIL   # 2048 n per block
NBLK = N // BLKN     # 8 blocks per batch

_CACHE = {}


def _build_module():
    from contextlib import ExitStack
    import concourse.tile as tile
    from concourse import bacc, mybir

    nc = bacc.Bacc("TRN2", target_bir_lowering=False, debug=False, num_devices=NCORES)
    bf = mybir.dt.bfloat16
    f32 = mybir.dt.float32

    x_d = nc.dram_tensor("x", [BPC, D, N], bf, kind="ExternalInput").ap()
    # xt[b, p, gi, d] = x[b, d, gi*128 + p]
    xt_d = nc.dram_tensor("xt", [BPC, 128, N // TILN, D + 1], bf, kind="ExternalInput").ap()
    # xsqc[b, p, sib, j] = x_sq[b, n],  n = sib*4096 + j*128 + p
    xsqc_d = nc.dram_tensor("xsqc", [BPC, 128, N // 4096, 32], f32, kind="ExternalInput").ap()
    # wmrep[p, j*K+k] = w[k] - maxs   (f32: bf16 would lose ~0.25 abs on wm*xsq)
    wmrep_d = nc.dram_tensor("wmrep", [128, 32 * K], f32, kind="ExternalInput").ap()
    # wcsqrep[0, j*K+k] = w[k]*csq[k]
    wcsqr_d = nc.dram_tensor("wcsqrep", [1, 32 * K], bf, kind="ExternalInput").ap()
    w1_d = nc.dram_tensor("w1", [D, K], bf, kind="ExternalInput").ap()
    oute_d = nc.dram_tensor("out_e", [BPC, K, 2, D + 1], f32, kind="ExternalOutput").ap()

    with tile.TileContext(nc) as tc, ExitStack() as ctx:
        cpool = ctx.enter_context(tc.tile_pool(name="const", bufs=1))
        xpool = ctx.enter_context(tc.tile_pool(name="xblk", bufs=4))
        xtpool = ctx.enter_context(tc.tile_pool(name="xtblk", bufs=4))
        qpool = ctx.enter_context(tc.tile_pool(name="xsqb", bufs=2))
        ppool = ctx.enter_context(tc.tile_pool(name="pexp", bufs=3))
        npool = ctx.enter_context(tc.tile_pool(name="pnorm", bufs=3))
        vpool = ctx.enter_context(tc.tile_pool(name="small", bufs=4))
        ps_xc = ctx.enter_context(tc.tile_pool(name="ps_xc", bufs=2, space="PSUM"))
        ps_e = ctx.enter_context(tc.tile_pool(name="ps_e", bufs=2, space="PSUM"))

        w1_sb = cpool.tile([D, K], bf)
        nc.sync.dma_start(out=w1_sb[:], in_=w1_d[:, :])
        wmrep_sb = cpool.tile([128, 32 * K], f32)
        nc.sync.dma_start(out=wmrep_sb[:], in_=wmrep_d[:, :])
        wcsqr_sb = cpool.tile([1, 32 * K], bf)
        nc.sync.dma_start(out=wcsqr_sb[:], in_=wcsqr_d[:, :])
        ones1_sb = cpool.tile([1, 128], bf)
        nc.vector.memset(ones1_sb[:], 1.0)

        NSUP = 2                 # blocks per superblock load
        SUPN = BLKN * NSUP       # 4096 n per load chunk
        NSB = NBLK // NSUP       # superblocks per batch
        TPS = NTIL * NSUP        # 32 tiles per superblock

        # Software pipeline: mm2s of superblock s are emitted after the
        # softmax chain of superblock s+1, so the PE hides the chain latency.
        pending = []  # (b, sup_in_batch, pn_sb, xt_sb)
        psum_es = {}
        g_bs = {}
        first_mm2 = {}

        def emit_mm2s(b, sib, pn_sb, xt_sb):
            pe0, pe1 = psum_es[b]
            ff = first_mm2[b]
            for i in range(TPS):
                pp = i % 2
                nc.tensor.matmul(
                    (pe0, pe1)[pp][:],
                    lhsT=pn_sb[:, K * i : K * (i + 1)],
                    rhs=xt_sb[:, i, :],
                    start=ff[pp],
                    stop=(sib == NSB - 1 and i >= TPS - 2),
                )
                ff[pp] = False
            if sib == NSB - 1:
                e_sb = vpool.tile([K, 2, D + 1], f32, tag="e_out")
                nc.vector.tensor_copy(e_sb[:, 0, :], pe0[:])
                nc.vector.tensor_copy(e_sb[:, 1, :], pe1[:])
                nc.sync.dma_start(out=oute_d[b], in_=e_sb[:])

        for gsup in range(BPC * NSB):
            b, sib = divmod(gsup, NSB)
            if sib == 0:
                xsq_b = qpool.tile([128, (N // 4096) * 32], f32, name=f"xsq_b{b}")
                nc.sync.dma_start(out=xsq_b[:], in_=xsqc_d[b].rearrange("p s j -> p (s j)"))
                t1_b = qpool.tile([128, N // 128 * K], f32, name=f"t1_b{b}", tag="t1b")
                nc.vector.tensor_tensor(
                    t1_b[:].rearrange("p (s j k) -> p s j k", j=32, k=K),
                    wmrep_sb[:].rearrange("p (j k) -> p j k", k=K)[:, None, :, :]
                    .broadcast_to([128, NBLK // NSUP, 32, K]),
                    xsq_b[:].rearrange("p (s j) -> p s j", j=32)[:, :, :, None]
                    .broadcast_to([128, NBLK // NSUP, 32, K]),
                    op=mybir.AluOpType.mult,
                )
                g_b = qpool.tile([128, N // 128 * K], bf, name=f"g_b{b}", tag="gb")
                nc.scalar.activation(
                    g_b[:], t1_b[:], mybir.ActivationFunctionType.Exp
                )
                g_bs[b] = g_b
                psum_es[b] = (
                    ps_e.tile([K, D + 1], f32, tag="pe0", name=f"psum_e0_b{b}"),
                    ps_e.tile([K, D + 1], f32, tag="pe1", name=f"psum_e1_b{b}"),
                )
                first_mm2[b] = [True, True]
            soff = sib * SUPN
            x_sb = xpool.tile([D, SUPN], bf)
            nc.sync.dma_start(out=x_sb[:], in_=x_d[b][:, soff : soff + SUPN])
            xt_sb = xtpool.tile([128, TPS, D + 1], bf)
            nc.scalar.dma_start(
                out=xt_sb[:], in_=xt_d[b][:, sib * TPS : (sib + 1) * TPS, :]
            )
            psum_xc = ps_xc.tile([128, TPS * K], f32)
            for h in range(2):
                nc.tensor.matmul(
                    psum_xc[:, 512 * h : 512 * (h + 1)],
                    lhsT=ones1_sb[:],
                    rhs=wcsqr_sb[:, 512 * h : 512 * (h + 1)],
                    start=True,
                    stop=False,
                    skip_group_check=True,
                )
            for i in range(TPS):
                nc.tensor.matmul(
                    psum_xc[:, K * i : K * (i + 1)],
                    lhsT=x_sb[:, TILN * i : TILN * (i + 1)],
                    rhs=w1_sb[:, :],
                    start=False,
                    stop=True,
                    skip_group_check=True,
                )

            pe_sb = ppool.tile([128, TPS * K], bf, tag="pexp")
            nc.scalar.activation(
                pe_sb[:], psum_xc[:], mybir.ActivationFunctionType.Exp
            )
            p_sb = ppool.tile([128, TPS * K], bf, tag="p")
            nc.gpsimd.tensor_mul(
                p_sb[:],
                pe_sb[:],
                g_bs[b][:, sib * TPS * K : (sib + 1) * TPS * K],
            )
            p3 = p_sb[:].rearrange("p (i k) -> p i k", k=K)
            s_sb = vpool.tile([128, TPS], f32, tag="s")
            nc.vector.reduce_sum(s_sb[:], p3, axis=mybir.AxisListType.X)
            sinv_sb = vpool.tile([128, TPS], f32, tag="sinv")
            nc.vector.reciprocal(sinv_sb[:], s_sb[:])
            pn_sb = npool.tile([128, TPS * K], bf, tag="pn")
            nc.vector.tensor_tensor(
                pn_sb[:].rearrange("p (i k) -> p i k", k=K),
                p3,
                sinv_sb[:].broadcast_to([128, TPS, K]),
                op=mybir.AluOpType.mult,
            )

            pending.append((b, sib, pn_sb, xt_sb))
            if len(pending) > 1:
                emit_mm2s(*pending.pop(0))

        while pending:
            emit_mm2s(*pending.pop(0))

    nc.compile()
    return nc


def _get_module():
    if "nc" not in _CACHE:
        _CACHE["nc"] = _build_module()
    return _CACHE["nc"]


def _host_prep(x, codewords, scale):
    x = np.asarray(x, dtype=np.float32)
    c = np.asarray(codewords, dtype=np.float32)
    s = np.asarray(scale, dtype=np.float32)

    w = -s                           # (K,) in (0, 1)
    maxs = float(w.max())
    w1 = (-2.0 * (w[:, None] * c)).T.astype(BF16)           # (D, K)
    wm = w - maxs                                           # (K,) <= 0
    wcsq = w * (c * c).sum(axis=1)                          # (K,)

    xf = x.reshape(B, D, N)
    xsq = np.einsum("bdn,bdn->bn", xf, xf)                  # (B, N) fp32
    # xsqc[b, p, s, j] = xsq[b, s*4096 + j*128 + p]
    xsqc = np.ascontiguousarray(
        xsq.reshape(B, N // 4096, 32, 128).transpose(0, 3, 1, 2)
    )                                                       # (B, 128, N/4096, 32) f32
    wmrep = np.ascontiguousarray(
        np.broadcast_to(np.tile(wm, 32)[None, :], (128, 32 * K))
    ).astype(np.float32)
    wcsqrep = np.tile(wcsq, 32)[None, :].astype(BF16)       # (1, 32*K)

    xb = xf.astype(BF16)                                    # (B, D, N)
    # xt[b, p, gi, d] = xf[b, d, gi*128 + p];  xt[..., D] = 1.0 (fused colsum column)
    xt = np.ones((B, N // TILN, TILN, D + 1), dtype=BF16)
    xt[:, :, :, :D] = xf.transpose(0, 2, 1).reshape(B, N // TILN, TILN, D).astype(BF16)
    xt = np.ascontiguousarray(xt.transpose(0, 2, 1, 3))     # (B, 128, N/128, D+1)
    return xb, xt, xsqc, wmrep, wcsqrep, w1


def make_in_maps(x, codewords, scale):
    xb, xt, xsqc, wmrep, wcsqrep, w1 = _host_prep(x, codewords, scale)
    in_maps = []
    for ci in range(NCORES):
        sl = slice(BPC * ci, BPC * (ci + 1))
        in_maps.append(
            {
                "x": np.ascontiguousarray(xb[sl]),
                "xt": np.ascontiguousarray(xt[sl]),
                "xsqc": np.ascontiguousarray(xsqc[sl]),
                "wmrep": wmrep,
                "wcsqrep": wcsqrep,
                "w1": w1,
            }
        )
    return in_maps


def finish_output(results, codewords):
    c = np.asarray(codewords, dtype=np.float32)
    out = np.zeros((B, K * D), dtype=np.float32)
    for ci, r in enumerate(results):
        for bb in range(BPC):
            e_parts = r["out_e"][bb][:, 0, :] + r["out_e"][bb][:, 1, :]   # (K, D+1)
            e = e_parts[:, :D] - e_parts[:, D : D + 1] * c
            out[BPC * ci + bb] = e.reshape(-1)
    return out


def kernel(x, codewords, scale):
    from concourse.bass_utils import run_bass_kernel_spmd
    from concourse.bass_interp import get_hw_module

    nc = _get_module()
    in_maps = make_in_maps(x, codewords, scale)

    old_m = nc.m
    nc.m = get_hw_module(nc.m)
    try:
        res = run_bass_kernel_spmd(nc, in_maps, core_ids=list(range(NCORES)))
    finally:
        nc.m = old_m
    return finish_output(res.results, codewords)
